# revision 7
# baseline (speedup 1.0000x reference)
"""CBOW forward kernel for one TRN2 chip (8 NeuronCores), tensor-parallel on vocab.

Math (matches the reference):
    embed[b, c, :] = emb_W.T[contexts[b, c]] + emb_b          # gather
    out = embed.reshape(B, CTX*EMB) @ fc_W.T + fc_b           # [B, VOCAB]

Distribution: vocab dim sharded 8 ways (fc_W rows / fc_b / output columns).
contexts + emb table replicated; EVERY core gathers the full batch locally so
there are NO collectives (the old AllGather cost ~68us of PE idle).

Numerics: the big matmul runs in fp8 e4m3 with perf_mode=DoubleRow (2 fp8
weights per PE cell -> 2x bf16 throughput). Both operands are scaled by 2^9,
clipped to +-240 (TRN e4m3 max) and RNE-quantized; PSUM accumulates fp32; the
drain multiplies by 2^-18 and adds the exact f32 effective bias
fc_be = fc_W @ tile(emb_b, CTX) + fc_b. Measured end-to-end rel err vs the
f32 reference: 1.39e-2 (gate 2e-2). emb_b folding into fc_be is exact.

Per-core schedule:
  1. 4 batched indirect-DMA gathers (512 rows/call, bf16 table) -> raw16
     [128 batch, 8192] = full 2048x8 gather, one call per 512-batch chunk
  2. fc_W fp8 shard (6.3 MB) streamed into SBUF once (resident), 14 chunks
  3. per batch chunk bc: 16 PE transposes (bf16) + DVE copy-casts build the
     pair-interleaved fp8 moving operand embT8[kg][k%256 part, batch, pair];
     then 98 vocab tiles x 2 DoubleRow matmuls (K=512 = 2 groups of 256)
     into one PSUM bank each; scalar/vector alternate the scale+bias drain;
     contiguous 128KB output DMA per (bc, vocab tile).
"""

import os

import numpy as np
import ml_dtypes

import concourse.bacc as bacc
import concourse.bass as bass
import concourse.mybir as mybir
import concourse.tile as tile
from concourse.bass_utils import run_bass_kernel_spmd
from concourse.masks import make_identity

# Problem shape (hardcoded per harness contract).
VOCAB = 100000
CTX = 8
EMB = 64
BATCH = 2048
K = CTX * EMB            # 512 contraction dim
NCORES = 8
VSHARD = 12544           # 98 * 128, vocab cols per core (padded)
VPAD = VSHARD * NCORES   # 100352
NVT = VSHARD // 128      # 98 vocab tiles per core
VCHUNK = 7               # vocab tiles per fc DMA chunk
NCHUNK = NVT // VCHUNK   # 14
CHUNK_COLS = VCHUNK * 128  # 896
NBT = BATCH // 128       # 16 batch tiles
NBC = 4                  # batch chunks (512 each) — outer loop
MPB = NBT // NBC         # batch tiles per chunk

F32 = mybir.dt.float32
BF16 = mybir.dt.bfloat16
FP8 = mybir.dt.float8e4
I32 = mybir.dt.int32
OUT_DT = BF16

E4NP = ml_dtypes.float8_e4m3   # TRN FP8_EXP4 semantics (max 240, inf above)
BFNP = ml_dtypes.bfloat16

SE = 2.0 ** 9            # embedding scale (pow2: exact descale)
SW = 2.0 ** 9            # fc weight scale
DESCALE = 1.0 / (SE * SW)

_CACHE = {}


def _install_trace_hook():
    """Provide the missing antenv.axon_hooks module so trace=True works."""
    import sys
    import types

    try:
        if "antenv.axon_hooks" not in sys.modules:
            mod = types.ModuleType("antenv.axon_hooks")
            mod._hook = None
            mod.set_axon_ntff_profile_hook = lambda h: setattr(mod, "_hook", h)
            mod.get_axon_ntff_profile_hook = lambda: mod._hook
            sys.modules["antenv.axon_hooks"] = mod
            import antenv

            antenv.axon_hooks = mod
        mod = sys.modules["antenv.axon_hooks"]
        if mod.get_axon_ntff_profile_hook() is None:
            if "/root/.axon_site/trn_agent_boot" not in sys.path:
                sys.path.insert(0, "/root/.axon_site/trn_agent_boot")
            import trn_boot

            mod.set_axon_ntff_profile_hook(
                trn_boot._ntff_profile_via_ctypes("/opt/axon/libaxon_pjrt.so")
            )
        return True
    except Exception as e:  # pragma: no cover
        print(f"trace hook install failed: {type(e).__name__}: {e}")
        return False


def _build_nc():
    nc = bacc.Bacc(
        "TRN2", target_bir_lowering=False, debug=False, num_devices=NCORES
    )
    # idx_all[p, j] = contexts[(j//8)*128 + p, j%8]  (j = m*8+c), same all cores
    idx_all = nc.declare_dram_parameter("idx_all", [128, 128], I32, isOutput=False)
    emb_wt = nc.declare_dram_parameter("emb_wt", [VOCAB, EMB], BF16, isOutput=False)
    # fc_w[ci, i, kg, par, w]: e4m3( SW * fc_W.T[kg*256+par*128+i, shard v] )
    fc_w = nc.declare_dram_parameter(
        "fc_w", [NCHUNK, 128, 2, 2, CHUNK_COLS], FP8, isOutput=False
    )
    fc_be = nc.declare_dram_parameter("fc_be", [128, NVT], F32, isOutput=False)
    out = nc.declare_dram_parameter(
        "out", [NBC, VSHARD, 512], OUT_DT, isOutput=True
    )

    with tile.TileContext(nc) as tc:
        with (
            tc.tile_pool(name="const", bufs=1) as const,
            tc.tile_pool(name="tpsum", bufs=2, space="PSUM") as tps,
            tc.tile_pool(name="mpsum", bufs=6, space="PSUM") as mps,
            tc.tile_pool(name="outp", bufs=8) as outp,
        ):
            idx_sb = const.tile([128, 128], I32, tag="idx", name="idx_sb")
            nc.sync.dma_start(out=idx_sb[:], in_=idx_all[:])
            fcbe_sb = const.tile([128, NVT], F32, tag="fcbe", name="fcbe_sb")
            nc.sync.dma_start(out=fcbe_sb[:], in_=fc_be[:])
            ident = const.tile([128, 128], BF16, tag="ident", name="ident")
            make_identity(nc, ident[:])
            # warm the ACT Identity table before the main loop needs it
            actwarm = const.tile([128, 1], F32, tag="actwarm", name="actwarm")
            nc.scalar.activation(
                out=actwarm[:],
                in_=fcbe_sb[:, 0:1],
                func=mybir.ActivationFunctionType.Identity,
                bias=fcbe_sb[:, 0:1],
            )

            # resident fc weights: [128 i, ci, kg, par, w] fp8 (49 KB/part)
            fcsb = const.tile(
                [128, NCHUNK, 2, 2, CHUNK_COLS], FP8, tag="fcsb", name="fcsb"
            )
            for ci in range(NCHUNK):
                nc.sync.dma_start(out=fcsb[:, ci], in_=fc_w[ci])

            # full-batch gather: raw16[p, j*64+e] = emb_wt[idx[p, j], e].
            # HW honors only ONE offset column per indirect call (multi-col
            # offset APs gather consecutive rows — verified on silicon), so
            # issue 128 single-column calls; bc0 needs only the first 32.
            raw16 = const.tile([128, NBT * K], BF16, tag="raw16", name="raw16")
            for j in range(NBT * CTX):
                nc.gpsimd.indirect_dma_start(
                    out=raw16[:, j * EMB : (j + 1) * EMB],
                    out_offset=None,
                    in_=emb_wt[:],
                    in_offset=bass.IndirectOffsetOnAxis(
                        ap=idx_sb[:, j : j + 1], axis=0
                    ),
                )

            # pair-interleaved fp8 moving operand:
            # embT8[kg][i, n, par] = e4m3(embed_scaled[kg*256+par*128+i, n])
            embT8 = [
                const.tile([128, BATCH, 2], FP8, tag=f"embT8{g}", name=f"embT8{g}")
                for g in range(2)
            ]

            def emit_transposes(bc):
                """PE transpose + DVE copy-cast building embT8 cols (m0,m1)."""
                for ml in range(2):
                    m = bc * MPB + ml
                    for kb in range(4):
                        ps = tps.tile([128, 1024], BF16, tag="tps", name="tps")
                        nc.tensor.transpose(
                            ps[:, 0:128],
                            raw16[:, m * K + kb * 128 : m * K + (kb + 1) * 128],
                            ident[:],
                        )
                        kg, par = kb // 2, kb % 2
                        nc.vector.tensor_copy(
                            out=embT8[kg][:, m * 128 : (m + 1) * 128, par],
                            in_=ps[:, 0:128],
                        )

            emit_transposes(0)
            # passes: bc0 split into two half-batch passes (compute starts
            # after only 16 gather calls), then full 512-batch passes.
            # Each pass interleaves the NEXT pass's transposes into its tail.
            passes = [
                (0, 0, 256, [(0, kb) for kb in range(4)] + [(1, kb) for kb in range(4)]),
                (0, 256, 512, [(2, kb) for kb in range(4)] + [(3, kb) for kb in range(4)]),
                (1, 0, 512, [(m, kb) for m in (4, 5, 6, 7) for kb in range(4)]),
                (2, 0, 512, [(m, kb) for m in (8, 9, 10, 11) for kb in range(4)]),
                (3, 0, 512, [(m, kb) for m in (12, 13, 14, 15) for kb in range(4)]),
            ]
            # pass p's tail prefetches pass p+1's transposes
            for pi, (bc, lo, hi, _tps) in enumerate(passes):
                csz = hi - lo
                nxt = passes[pi + 1][3] if pi + 1 < len(passes) else []
                ntp = len(nxt)
                tp_start = NVT - 3 * ntp - 2 if ntp else NVT + 1
                for v in range(NVT):
                    ci, vt = v // VCHUNK, v % VCHUNK
                    psm = mps.tile([128, 512], F32, tag="mps", name="mps")
                    for kg in range(2):
                        nc.tensor.matmul(
                            out=psm[:, 0:csz],
                            lhsT=fcsb[:, ci, kg, :, vt * 128 : (vt + 1) * 128],
                            rhs=embT8[kg][
                                :, bc * 512 + lo : bc * 512 + hi, :
                            ].rearrange("p n t -> p t n"),
                            start=(kg == 0),
                            stop=(kg == 1),
                            perf_mode=mybir.MatmulPerfMode.DoubleRow,
                        )
                    if v % 2 == 0:
                        osb2 = outp.tile(
                            [128, 2, 512], OUT_DT, tag="osb2", name="osb2"
                        )
                        nc.scalar.activation(
                            out=osb2[:, 0, 0:csz],
                            in_=psm[:, 0:csz],
                            func=mybir.ActivationFunctionType.Identity,
                            bias=fcbe_sb[:, v : v + 1],
                            scale=DESCALE,
                        )
                    else:
                        nc.vector.tensor_scalar(
                            out=osb2[:, 1, 0:csz],
                            in0=psm[:, 0:csz],
                            scalar1=DESCALE,
                            scalar2=fcbe_sb[:, v : v + 1],
                            op0=mybir.AluOpType.mult,
                            op1=mybir.AluOpType.add,
                        )
                        nc.sync.dma_start(
                            out=out[
                                bc, (v - 1) * 128 : (v + 1) * 128, lo:hi
                            ].rearrange("(i p) c -> p i c", p=128),
                            in_=osb2[:, 0:2, 0:csz],
                        )
                    # trickle next pass's transposes through this pass's tail
                    if ntp and v >= tp_start and (v - tp_start) % 3 == 0:
                        ti = (v - tp_start) // 3
                        if ti < ntp:
                            m, kb = nxt[ti]
                            ps = tps.tile(
                                [128, 1024], BF16, tag="tps", name="tps"
                            )
                            nc.tensor.transpose(
                                ps[:, 0:128],
                                raw16[
                                    :,
                                    m * K + kb * 128 : m * K + (kb + 1) * 128,
                                ],
                                ident[:],
                            )
                            kg, par = kb // 2, kb % 2
                            nc.vector.tensor_copy(
                                out=embT8[kg][:, m * 128 : (m + 1) * 128, par],
                                in_=ps[:, 0:128],
                            )
    nc.compile()
    return nc


def _prep_inputs(contexts, emb_W, emb_b, fc_W, fc_b):
    contexts = np.asarray(contexts)
    emb_W = np.asarray(emb_W, dtype=np.float32)
    emb_b = np.asarray(emb_b, dtype=np.float32)
    fc_W = np.asarray(fc_W, dtype=np.float32)
    fc_b = np.asarray(fc_b, dtype=np.float32)

    # idx_all[p, j] = contexts[(j//8)*128 + p, j%8]
    idx2d = (
        contexts.astype(np.int64).reshape(NBT, 128, CTX).transpose(0, 2, 1)
        .reshape(NBT * CTX, 128)
    )
    idx_all = np.ascontiguousarray(idx2d.T.astype(np.int32))

    # scaled bf16 embedding table (device casts bf16 -> e4m3 during drain copy)
    emb_wt = np.ascontiguousarray(
        np.clip(emb_W.T * SE, -240.0, 240.0).astype(BFNP)
    )

    # effective bias: fc_be = fc_W @ tile(emb_b, CTX) + fc_b  (exact, padded)
    emb_b_t = np.tile(emb_b, CTX)
    fc_be_full = (
        fc_W.astype(np.float64) @ emb_b_t.astype(np.float64)
        + fc_b.astype(np.float64)
    ).astype(np.float32)
    fc_be_pad = np.zeros(VPAD, dtype=np.float32)
    fc_be_pad[:VOCAB] = fc_be_full

    # fc_W.T scaled/quantized to e4m3, padded to VPAD cols
    fcT = np.zeros((K, VPAD), dtype=np.float32)
    fcT[:, :VOCAB] = fc_W.T
    fcq = np.clip(fcT * SW, -240.0, 240.0).astype(E4NP)

    in_maps = []
    for s in range(NCORES):
        shard = fcq[:, s * VSHARD : (s + 1) * VSHARD]
        # [k=kg*256+par*128+i, v=ci*896+w] -> [ci, i, kg, par, w]
        fc_host = np.ascontiguousarray(
            shard.reshape(2, 2, 128, NCHUNK, CHUNK_COLS).transpose(3, 2, 0, 1, 4)
        )
        be = np.ascontiguousarray(
            fc_be_pad[s * VSHARD : (s + 1) * VSHARD].reshape(NVT, 128).T
        )
        in_maps.append(
            {"idx_all": idx_all, "emb_wt": emb_wt, "fc_w": fc_host, "fc_be": be}
        )
    return in_maps


def kernel(contexts, emb_W, emb_b, fc_W, fc_b):
    if "nc" not in _CACHE:
        _CACHE["nc"] = _build_nc()
    nc = _CACHE["nc"]
    in_maps = _prep_inputs(contexts, emb_W, emb_b, fc_W, fc_b)
    trace = bool(int(os.environ.get("KERNEL_TRACE", "0")))
    if trace:
        trace = _install_trace_hook()
    res = run_bass_kernel_spmd(
        nc, in_maps, core_ids=list(range(NCORES)), trace=trace
    )
    _CACHE["last_exec_time_ns"] = res.exec_time_ns
    # out[s][bc, v*128+r, j] = logits[bc*512+j, s*VSHARD + v*128+r]
    full = np.empty((BATCH, VPAD), dtype=np.float32)
    for s, r in enumerate(res.results):
        o = np.asarray(r["out"]).astype(np.float32)  # [NBC, VSHARD, 512]
        for bc in range(NBC):
            full[bc * 512 : (bc + 1) * 512, s * VSHARD : (s + 1) * VSHARD] = o[
                bc
            ].T
    return np.ascontiguousarray(full[:, :VOCAB])


# revision 8
# speedup vs baseline: 1.4982x; 1.4982x over previous
"""CBOW forward kernel for one TRN2 chip (8 NeuronCores), tensor-parallel on vocab.

Math (matches the reference):
    embed[b, c, :] = emb_W.T[contexts[b, c]] + emb_b          # gather
    out = embed.reshape(B, CTX*EMB) @ fc_W.T + fc_b           # [B, VOCAB]

Distribution: vocab dim sharded 8 ways (fc_W rows / fc_b / output columns).
contexts + emb table replicated; EVERY core gathers the full batch locally so
there are NO collectives (the old AllGather cost ~68us of PE idle).

Numerics: the big matmul runs in fp8 e4m3 with perf_mode=DoubleRow (2 fp8
weights per PE cell -> 2x bf16 throughput). Both operands are scaled by 2^9,
clipped to +-240 (TRN e4m3 max) and RNE-quantized; PSUM accumulates fp32; the
drain multiplies by 2^-18 and adds the exact f32 effective bias
fc_be = fc_W @ tile(emb_b, CTX) + fc_b. Measured end-to-end rel err vs the
f32 reference: 1.39e-2 (gate 2e-2). emb_b folding into fc_be is exact.

Per-core schedule:
  1. 4 batched indirect-DMA gathers (512 rows/call, bf16 table) -> raw16
     [128 batch, 8192] = full 2048x8 gather, one call per 512-batch chunk
  2. fc_W fp8 shard (6.3 MB) streamed into SBUF once (resident), 14 chunks
  3. per batch chunk bc: 16 PE transposes (bf16) + DVE copy-casts build the
     pair-interleaved fp8 moving operand embT8[kg][k%256 part, batch, pair];
     then 98 vocab tiles x 2 DoubleRow matmuls (K=512 = 2 groups of 256)
     into one PSUM bank each; scalar/vector alternate the scale+bias drain;
     contiguous 128KB output DMA per (bc, vocab tile).
"""

import os

import numpy as np
import ml_dtypes

import concourse.bacc as bacc
import concourse.bass as bass
import concourse.mybir as mybir
import concourse.tile as tile
from concourse.bass_utils import run_bass_kernel_spmd
from concourse.masks import make_identity

# Problem shape (hardcoded per harness contract).
VOCAB = 100000
CTX = 8
EMB = 64
BATCH = 2048
K = CTX * EMB            # 512 contraction dim
NCORES = 8
VSHARD = 12544           # 98 * 128, vocab cols per core (padded)
VPAD = VSHARD * NCORES   # 100352
NVT = VSHARD // 128      # 98 vocab tiles per core
VCHUNK = 7               # vocab tiles per fc DMA chunk
NCHUNK = NVT // VCHUNK   # 14
CHUNK_COLS = VCHUNK * 128  # 896
NBT = BATCH // 128       # 16 batch tiles
NBC = 4                  # batch chunks (512 each) — outer loop
MPB = NBT // NBC         # batch tiles per chunk

F32 = mybir.dt.float32
BF16 = mybir.dt.bfloat16
FP8 = mybir.dt.float8e4
I32 = mybir.dt.int32
OUT_DT = BF16

E4NP = ml_dtypes.float8_e4m3   # TRN FP8_EXP4 semantics (max 240, inf above)
BFNP = ml_dtypes.bfloat16

SE = 2.0 ** 9            # embedding scale (pow2: exact descale)
SW = 2.0 ** 9            # fc weight scale
DESCALE = 1.0 / (SE * SW)

_CACHE = {}


def _install_trace_hook():
    """Provide the missing antenv.axon_hooks module so trace=True works."""
    import sys
    import types

    try:
        if "antenv.axon_hooks" not in sys.modules:
            mod = types.ModuleType("antenv.axon_hooks")
            mod._hook = None
            mod.set_axon_ntff_profile_hook = lambda h: setattr(mod, "_hook", h)
            mod.get_axon_ntff_profile_hook = lambda: mod._hook
            sys.modules["antenv.axon_hooks"] = mod
            import antenv

            antenv.axon_hooks = mod
        mod = sys.modules["antenv.axon_hooks"]
        if mod.get_axon_ntff_profile_hook() is None:
            if "/root/.axon_site/trn_agent_boot" not in sys.path:
                sys.path.insert(0, "/root/.axon_site/trn_agent_boot")
            import trn_boot

            mod.set_axon_ntff_profile_hook(
                trn_boot._ntff_profile_via_ctypes("/opt/axon/libaxon_pjrt.so")
            )
        return True
    except Exception as e:  # pragma: no cover
        print(f"trace hook install failed: {type(e).__name__}: {e}")
        return False


def _build_nc():
    nc = bacc.Bacc(
        "TRN2", target_bir_lowering=False, debug=False, num_devices=NCORES
    )
    # idx_all[p, j] = contexts[(j//8)*128 + p, j%8]  (j = m*8+c), same all cores
    idx_all = nc.declare_dram_parameter("idx_all", [128, 128], I32, isOutput=False)
    emb_wt = nc.declare_dram_parameter("emb_wt", [VOCAB, EMB], BF16, isOutput=False)
    # fc_w[ci, i, kg, par, w]: e4m3( SW * fc_W.T[kg*256+par*128+i, shard v] )
    fc_w = nc.declare_dram_parameter(
        "fc_w", [NCHUNK, 128, 2, 2, CHUNK_COLS], FP8, isOutput=False
    )
    fc_be = nc.declare_dram_parameter("fc_be", [128, NVT], F32, isOutput=False)
    out = nc.declare_dram_parameter(
        "out", [NBC, VSHARD, 512], OUT_DT, isOutput=True
    )

    with tile.TileContext(nc) as tc:
        with (
            tc.tile_pool(name="const", bufs=1) as const,
            tc.tile_pool(name="tpsum", bufs=2, space="PSUM") as tps,
            tc.tile_pool(name="mpsum", bufs=6, space="PSUM") as mps,
            tc.tile_pool(name="outp", bufs=8) as outp,
        ):
            idx_sb = const.tile([128, 128], I32, tag="idx", name="idx_sb")
            nc.sync.dma_start(out=idx_sb[:], in_=idx_all[:])
            fcbe_sb = const.tile([128, NVT], F32, tag="fcbe", name="fcbe_sb")
            nc.sync.dma_start(out=fcbe_sb[:], in_=fc_be[:])
            ident = const.tile([128, 128], BF16, tag="ident", name="ident")
            make_identity(nc, ident[:])
            # warm the ACT Identity table before the main loop needs it
            actwarm = const.tile([128, 1], F32, tag="actwarm", name="actwarm")
            nc.scalar.activation(
                out=actwarm[:],
                in_=fcbe_sb[:, 0:1],
                func=mybir.ActivationFunctionType.Identity,
                bias=fcbe_sb[:, 0:1],
            )

            # resident fc weights: [128 i, ci, kg, par, w] fp8 (49 KB/part)
            fcsb = const.tile(
                [128, NCHUNK, 2, 2, CHUNK_COLS], FP8, tag="fcsb", name="fcsb"
            )
            for ci in range(NCHUNK):
                nc.sync.dma_start(out=fcsb[:, ci], in_=fc_w[ci])

            # full-batch gather: raw16[p, j*64+e] = emb_wt[idx[p, j], e].
            # HW honors only ONE offset column per indirect call (multi-col
            # offset APs gather consecutive rows — verified on silicon), so
            # issue 128 single-column calls; bc0 needs only the first 32.
            raw16 = const.tile([128, NBT * K], BF16, tag="raw16", name="raw16")
            for j in range(NBT * CTX):
                nc.gpsimd.indirect_dma_start(
                    out=raw16[:, j * EMB : (j + 1) * EMB],
                    out_offset=None,
                    in_=emb_wt[:],
                    in_offset=bass.IndirectOffsetOnAxis(
                        ap=idx_sb[:, j : j + 1], axis=0
                    ),
                )

            # pair-interleaved fp8 moving operand:
            # embT8[kg][i, n, par] = e4m3(embed_scaled[kg*256+par*128+i, n])
            embT8 = [
                const.tile([128, BATCH, 2], FP8, tag=f"embT8{g}", name=f"embT8{g}")
                for g in range(2)
            ]

            def emit_transposes(bc):
                """PE transpose + DVE copy-cast building embT8 cols for bc."""
                for ml in range(MPB):
                    m = bc * MPB + ml
                    for kb in range(4):
                        ps = tps.tile([128, 1024], BF16, tag="tps", name="tps")
                        nc.tensor.transpose(
                            ps[:, 0:128],
                            raw16[:, m * K + kb * 128 : m * K + (kb + 1) * 128],
                            ident[:],
                        )
                        kg, par = kb // 2, kb % 2
                        nc.vector.tensor_copy(
                            out=embT8[kg][:, m * 128 : (m + 1) * 128, par],
                            in_=ps[:, 0:128],
                        )

            emit_transposes(0)
            # full 512-batch passes; pass p's tail prefetches pass p+1's
            # 16 transposes late enough that their gathers are complete
            for bc in range(NBC):
                ntp = 16 if bc + 1 < NBC else 0
                tp_start = 66
                for v in range(NVT):
                    ci, vt = v // VCHUNK, v % VCHUNK
                    psm = mps.tile([128, 512], F32, tag="mps", name="mps")
                    for kg in range(2):
                        nc.tensor.matmul(
                            out=psm[:],
                            lhsT=fcsb[:, ci, kg, :, vt * 128 : (vt + 1) * 128],
                            rhs=embT8[kg][
                                :, bc * 512 : (bc + 1) * 512, :
                            ].rearrange("p n t -> p t n"),
                            start=(kg == 0),
                            stop=(kg == 1),
                            perf_mode=mybir.MatmulPerfMode.DoubleRow,
                        )
                    if v % 4 == 0:
                        osb4 = outp.tile(
                            [128, 4, 512], OUT_DT, tag="osb4", name="osb4"
                        )
                    if v % 2 == 0:
                        nc.scalar.activation(
                            out=osb4[:, v % 4, :],
                            in_=psm[:],
                            func=mybir.ActivationFunctionType.Identity,
                            bias=fcbe_sb[:, v : v + 1],
                            scale=DESCALE,
                        )
                    else:
                        nc.vector.tensor_scalar(
                            out=osb4[:, v % 4, :],
                            in0=psm[:],
                            scalar1=DESCALE,
                            scalar2=fcbe_sb[:, v : v + 1],
                            op0=mybir.AluOpType.mult,
                            op1=mybir.AluOpType.add,
                        )
                    if v % 4 == 3 or v == NVT - 1:
                        cnt = v % 4 + 1
                        nc.sync.dma_start(
                            out=out[
                                bc, (v - cnt + 1) * 128 : (v + 1) * 128, :
                            ].rearrange("(i p) c -> p i c", p=128),
                            in_=osb4[:, 0:cnt, :],
                        )
                    # trickle next chunk's 16 transposes through the tail
                    if ntp and v >= tp_start and (v - tp_start) % 2 == 0:
                        ti = (v - tp_start) // 2
                        if ti < ntp:
                            m = (bc + 1) * MPB + ti // 4
                            kb = ti % 4
                            ps = tps.tile(
                                [128, 1024], BF16, tag="tps", name="tps"
                            )
                            nc.tensor.transpose(
                                ps[:, 0:128],
                                raw16[
                                    :,
                                    m * K + kb * 128 : m * K + (kb + 1) * 128,
                                ],
                                ident[:],
                            )
                            kg, par = kb // 2, kb % 2
                            nc.vector.tensor_copy(
                                out=embT8[kg][:, m * 128 : (m + 1) * 128, par],
                                in_=ps[:, 0:128],
                            )
    nc.compile()
    return nc


def _prep_inputs(contexts, emb_W, emb_b, fc_W, fc_b):
    contexts = np.asarray(contexts)
    emb_W = np.asarray(emb_W, dtype=np.float32)
    emb_b = np.asarray(emb_b, dtype=np.float32)
    fc_W = np.asarray(fc_W, dtype=np.float32)
    fc_b = np.asarray(fc_b, dtype=np.float32)

    # idx_all[p, j] = contexts[(j//8)*128 + p, j%8]
    idx2d = (
        contexts.astype(np.int64).reshape(NBT, 128, CTX).transpose(0, 2, 1)
        .reshape(NBT * CTX, 128)
    )
    idx_all = np.ascontiguousarray(idx2d.T.astype(np.int32))

    # scaled bf16 embedding table (device casts bf16 -> e4m3 during drain copy)
    emb_wt = np.ascontiguousarray(
        np.clip(emb_W.T * SE, -240.0, 240.0).astype(BFNP)
    )

    # effective bias: fc_be = fc_W @ tile(emb_b, CTX) + fc_b  (exact, padded)
    emb_b_t = np.tile(emb_b, CTX)
    fc_be_full = (
        fc_W.astype(np.float64) @ emb_b_t.astype(np.float64)
        + fc_b.astype(np.float64)
    ).astype(np.float32)
    fc_be_pad = np.zeros(VPAD, dtype=np.float32)
    fc_be_pad[:VOCAB] = fc_be_full

    # fc_W.T scaled/quantized to e4m3, padded to VPAD cols
    fcT = np.zeros((K, VPAD), dtype=np.float32)
    fcT[:, :VOCAB] = fc_W.T
    fcq = np.clip(fcT * SW, -240.0, 240.0).astype(E4NP)

    in_maps = []
    for s in range(NCORES):
        shard = fcq[:, s * VSHARD : (s + 1) * VSHARD]
        # [k=kg*256+par*128+i, v=ci*896+w] -> [ci, i, kg, par, w]
        fc_host = np.ascontiguousarray(
            shard.reshape(2, 2, 128, NCHUNK, CHUNK_COLS).transpose(3, 2, 0, 1, 4)
        )
        be = np.ascontiguousarray(
            fc_be_pad[s * VSHARD : (s + 1) * VSHARD].reshape(NVT, 128).T
        )
        in_maps.append(
            {"idx_all": idx_all, "emb_wt": emb_wt, "fc_w": fc_host, "fc_be": be}
        )
    return in_maps


def kernel(contexts, emb_W, emb_b, fc_W, fc_b):
    if "nc" not in _CACHE:
        _CACHE["nc"] = _build_nc()
    nc = _CACHE["nc"]
    in_maps = _prep_inputs(contexts, emb_W, emb_b, fc_W, fc_b)
    trace = bool(int(os.environ.get("KERNEL_TRACE", "0")))
    if trace:
        trace = _install_trace_hook()
    res = run_bass_kernel_spmd(
        nc, in_maps, core_ids=list(range(NCORES)), trace=trace
    )
    _CACHE["last_exec_time_ns"] = res.exec_time_ns
    # out[s][bc, v*128+r, j] = logits[bc*512+j, s*VSHARD + v*128+r]
    full = np.empty((BATCH, VPAD), dtype=np.float32)
    for s, r in enumerate(res.results):
        o = np.asarray(r["out"]).astype(np.float32)  # [NBC, VSHARD, 512]
        for bc in range(NBC):
            full[bc * 512 : (bc + 1) * 512, s * VSHARD : (s + 1) * VSHARD] = o[
                bc
            ].T
    return np.ascontiguousarray(full[:, :VOCAB])
